# revision 14
# baseline (speedup 1.0000x reference)
"""Trainium2 Bass kernel for nn_DiffusionLayer (N=8192, D=128), 8-core SPMD.

Computation:
    t = relu(Z @ W1 + b1) @ W2 + b2      # [N, D]  (the MLP "transform")
    S = softmax(t @ t.T, axis=1)         # [N, N]
    out = Z + TAU * (S @ Z - Z)

Sharding (per the problem's hint): output rows split across 8 NeuronCores;
each core holds its 1024-row block and computes its S row-block against a
replicated transform_Z (t) and Z — flash-attention-style sequence
parallelism. t is computed once on the host (0.8% of total FLOPs — the
hint's "all-gathered transform_Z") and replicated to all cores as a bf16
hi+lo pair, which reconstructs to ~1e-7 relative accuracy; the O(N^2)
attention (99.2% of FLOPs) runs on device.

Device pipeline per core:
  - t^T via the DMA xbar transpose (2-byte only, hence the hi/lo pair) +
    one DVE add -> fp32r t^T in SBUF. No PE/PSUM involvement at all.
  - sim^T tiles [j-tile 128, i-chunk 256] = t[jt]^T' @ t_blk^T via fp32r
    matmuls (full speed, ~1.6e-4), grouped 6 j-tiles per 3-bank PSUM
    group so exp reads 1536-wide.
  - exp on ScalarE with a host-computed constant shift -C as the free
    activation bias (softmax is shift invariant; C keeps exp inside fp32
    range; row maxima >= ||t_i||^2 bound the denominator from below).
    Output E in bf16.
  - PV: E-slice^T @ [Zh | 1] in bf16, accumulated over all 64 j-tiles in
    PSUM; the appended ones column makes the softmax denominator fall
    out as output column 128. PV of chunk c-1 interleaves with sim of
    chunk c on the Tensor engine while ScalarE does exp.
  - normalize + residual on DVE per chunk, DMA out per chunk.
"""

import sys

sys.path.insert(0, "/opt/trn_rl_repo")

import numpy as np
import ml_dtypes
import orjson
from contextlib import ExitStack

import concourse.bass as bass
import concourse.tile as tile
from concourse import mybir
from concourse.bass_utils import run_bass_kernel_spmd

F32 = mybir.dt.float32
F32R = mybir.dt.float32r
BF16 = mybir.dt.bfloat16
BF = ml_dtypes.bfloat16

N, D = 8192, 128
NCORES = 8
BLK = N // NCORES  # 1024 rows per core
NT = N // 128  # 64 row tiles of full Z
NBT = BLK // 128  # 8 row tiles of the block
TAU = 0.1

CH = 256  # i-chunk width for sim/exp/PV
NCH = BLK // CH  # 4 chunks per core
GJ = 6  # j-tiles per sim PSUM group (3 banks)

# ---------------------------------------------------------------------------
# BIR post-pass: the walrus build in this image encodes at most one sync wait
# per instruction; Tile emits several on some instructions. Split excess
# waits onto preceding same-engine NoOp carriers (engines execute their
# stream in order, so this preserves semantics; NoOp stalls dispatch without
# flushing the engine pipeline).
_MAX_WAITS = 1


def _split_multiwaits(m: dict) -> bool:
    changed = False
    counter = [0]

    def fresh_name():
        counter[0] += 1
        return f"I-waitsplit-{counter[0]}"

    for fn in m.get("functions", []):
        for bb in fn.get("blocks", []):
            out = []
            for inst in bb.get("instructions", []):
                si = inst.get("sync_info") or {}
                waits = si.get("on_wait") or []
                if len(waits) > _MAX_WAITS:
                    changed = True
                    head, tail = waits[:-_MAX_WAITS], waits[-_MAX_WAITS:]
                    for i in range(0, len(head), _MAX_WAITS):
                        out.append(
                            {
                                "debug": inst.get("debug", 0),
                                "engine": inst["engine"],
                                "ins": [],
                                "name": fresh_name(),
                                "opcode": "NoOp",
                                "outs": [],
                                "sync_info": {
                                    "on_update": [],
                                    "on_wait": head[i : i + _MAX_WAITS],
                                },
                            }
                        )
                    si["on_wait"] = tail
                out.append(inst)
            bb["instructions"] = out
    return changed


def _patch_nc(nc):
    orig = nc.to_json_bytes

    def to_json_bytes_fixed():
        m = orjson.loads(orig())
        if _split_multiwaits(m):
            return orjson.dumps(m)
        return orig()

    nc.to_json_bytes = to_json_bytes_fixed
    return nc


# ---------------------------------------------------------------------------


def _build_nc(c_shift: float):
    nc = bass.Bass("TRN2", debug=False, num_devices=NCORES)

    Thd = nc.dram_tensor("Th", [N, D], BF16, kind="ExternalInput").ap()
    Tld = nc.dram_tensor("Tl", [N, D], BF16, kind="ExternalInput").ap()
    Tbhd = nc.dram_tensor("Tbh", [BLK, D], BF16, kind="ExternalInput").ap()
    Tbld = nc.dram_tensor("Tbl", [BLK, D], BF16, kind="ExternalInput").ap()
    Zhd = nc.dram_tensor("Zh", [N, D], BF16, kind="ExternalInput").ap()
    Zbd = nc.dram_tensor("Zb", [BLK, D], F32, kind="ExternalInput").ap()
    Od = nc.dram_tensor("O", [BLK, D], F32, kind="ExternalOutput").ap()

    Zhr = Zhd.rearrange("(t p) d -> p t d", p=128)  # [128, 64, 128]
    Zbr = Zbd.rearrange("(t p) d -> p t d", p=128)  # [128, 8, 128]
    Or = Od.rearrange("(t p) d -> p t d", p=128)

    with tile.TileContext(nc) as tc, ExitStack() as ctx:
        const = ctx.enter_context(tc.tile_pool(name="const", bufs=1))
        sb = ctx.enter_context(tc.tile_pool(name="sb", bufs=1))
        ebig = ctx.enter_context(tc.tile_pool(name="ebig", bufs=2))
        simps = ctx.enter_context(tc.tile_pool(name="simps", bufs=1, space="PSUM"))
        pvps = ctx.enter_context(tc.tile_pool(name="pvps", bufs=2, space="PSUM"))

        # ---- constants
        dummy = const.tile([128, 1], F32)
        nc.vector.memset(dummy[:], 0.0)
        dummy2 = const.tile([128, 1], F32)
        # preload the exp table set so the first real exp doesn't stall
        nc.scalar.activation(dummy2[:], dummy[:], mybir.ActivationFunctionType.Exp)
        cbias = const.tile([128, 1], F32)  # per-partition exp bias = -C
        nc.vector.memset(cbias[:], -c_shift)

        # ---- persistent SBUF tensors
        t_sb = sb.tile([128, N], F32R)  # t^T [d, N]
        tthi = sb.tile([128, N], BF16)
        ttlo = sb.tile([128, N], BF16)
        tb_sb = sb.tile([128, BLK], F32R)  # t_blk^T
        tbth = sb.tile([128, BLK], BF16)
        tbtl = sb.tile([128, BLK], BF16)
        zaug = sb.tile([128, NT, D + 1], BF16)  # [Zh | 1] row tiles
        zbn = sb.tile([128, NBT, 128], F32)  # Z block natural (residual)
        u_sb = sb.tile([128, NBT, D + 1], F32)  # unnormalized PV + denom
        o_sb = sb.tile([128, NBT, 128], F32)
        rec = sb.tile([128, NBT, 1], F32)

        # ---- all xbar transposes batched (Tile serializes on xbar-mode
        # transitions vs normal DMAs); block rows first — tb gates sim.
        nc.sync.dma_start_transpose(tbth[:], Tbhd)
        nc.sync.dma_start_transpose(tbtl[:], Tbld)
        for q in range(8):
            qs = slice(1024 * q, 1024 * (q + 1))
            nc.sync.dma_start_transpose(tthi[:, qs], Thd[qs, :])
            nc.sync.dma_start_transpose(ttlo[:, qs], Tld[qs, :])

        # ---- normal DMAs
        for q in range(4):
            nc.sync.dma_start(
                zaug[:, 16 * q : 16 * (q + 1), 0:D], Zhr[:, 16 * q : 16 * (q + 1), :]
            )
        nc.vector.memset(zaug[:, :, D : D + 1], 1.0)
        nc.sync.dma_start(zbn[:, 0:4, :], Zbr[:, 0:4, :])
        nc.sync.dma_start(zbn[:, 4:8, :], Zbr[:, 4:8, :])

        # ---- reconstruct fp32r t^T = hi + lo on DVE
        nc.vector.tensor_add(tb_sb[:], tbth[:], tbtl[:])
        for q in range(8):
            qs = slice(1024 * q, 1024 * (q + 1))
            nc.vector.tensor_add(t_sb[:, qs], tthi[:, qs], ttlo[:, qs])

        # ---- sim + exp + PV, chunked over i
        # One static 6-bank PSUM region. Chunk 0 ping-pongs its two 3-bank
        # halves (GJ=6 groups — no PV work exists yet to cover ACT-serial
        # stalls); later chunks use 12-jt rounds so each ACT exp reads 3072
        # elements and the per-instruction overhead halves (PE fills the
        # exp shadow with the previous chunk's PV matmuls).
        simtile = simps.tile([128, 2 * GJ, CH], F32, name="simtile")

        e_tiles = [None] * NCH

        def emit_sim_exp(c):
            ic = slice(CH * c, CH * (c + 1))
            e_sb = ebig.tile([128, NT, CH], BF16, tag="ebig", name=f"e_{c}")
            e_tiles[c] = e_sb
            gj = GJ if c == 0 else 2 * GJ
            go = 0
            half = 0
            while go < NT:
                gn = min(gj, NT - go)
                base = half * GJ if c == 0 else 0
                for k in range(gn):
                    jt = go + k
                    nc.tensor.matmul(
                        simtile[:, base + k, :],
                        t_sb[:, 128 * jt : 128 * (jt + 1)],
                        tb_sb[:, ic],
                        start=True,
                        stop=True,
                    )
                nc.scalar.activation(
                    e_sb[:, go : go + gn, :],
                    simtile[:, base : base + gn, :],
                    mybir.ActivationFunctionType.Exp,
                    bias=cbias[:],
                )
                go += gn
                half ^= 1

        def emit_pv(c):
            e_sb = e_tiles[c]
            for s in (2 * c, 2 * c + 1):
                si = (s % 2) * 128
                pv = pvps.tile([128, D + 1], F32, tag="ps", name=f"pv_{s}")
                for jt in range(NT):
                    nc.tensor.matmul(
                        pv[:],
                        e_sb[:, jt, si : si + 128],
                        zaug[:, jt, :],
                        start=(jt == 0),
                        stop=(jt == NT - 1),
                    )
                nc.vector.tensor_copy(u_sb[:, s, :], pv[:])

        def emit_out(c):
            # normalize + residual + store for chunk c's two row-slices
            sl = slice(2 * c, 2 * c + 2)
            nc.vector.reciprocal(rec[:, sl, :], u_sb[:, sl, D : D + 1])
            nc.vector.tensor_scalar_mul(rec[:, sl, :], rec[:, sl, :], TAU)
            for s in (2 * c, 2 * c + 1):
                nc.vector.tensor_scalar_mul(
                    u_sb[:, s, 0:D], u_sb[:, s, 0:D], rec[:, s, :]
                )
                nc.vector.scalar_tensor_tensor(
                    o_sb[:, s, :],
                    zbn[:, s, :],
                    1.0 - TAU,
                    u_sb[:, s, 0:D],
                    mybir.AluOpType.mult,
                    mybir.AluOpType.add,
                )
            nc.sync.dma_start(Or[:, sl, :], o_sb[:, sl, :])

        for c in range(NCH):
            emit_sim_exp(c)
            if c > 0:
                emit_pv(c - 1)
                emit_out(c - 1)
        emit_pv(NCH - 1)
        emit_out(NCH - 1)

    return _patch_nc(nc)


# ---------------------------------------------------------------------------

_CACHE = {}


def _get_nc(c_shift: float):
    key = round(float(c_shift), 3)
    if key not in _CACHE:
        _CACHE[key] = _build_nc(key)
    return _CACHE[key]


def prepare(Z, W1, b1, W2, b2):
    """Host-side prep: transform t, hi/lo splits, shift C, per-core maps."""
    Z = np.ascontiguousarray(np.asarray(Z, dtype=np.float32))
    W1 = np.ascontiguousarray(np.asarray(W1, dtype=np.float32))
    W2 = np.ascontiguousarray(np.asarray(W2, dtype=np.float32))
    b1 = np.asarray(b1, dtype=np.float32).reshape(1, D)
    b2 = np.asarray(b2, dtype=np.float32).reshape(1, D)

    t = (np.maximum(Z @ W1 + b1, 0.0) @ W2 + b2).astype(np.float32)
    Th = t.astype(BF)
    Tl = (t - Th.astype(np.float32)).astype(BF)
    Zh = Z.astype(BF)

    # constant softmax shift C: sim <= max||t||^2 (Cauchy-Schwarz), row
    # maxima >= diag = ||t_i||^2, so this window keeps exp in fp32 range
    # and the denominators in normal range.
    d2 = np.einsum("nd,nd->n", t, t)
    c_shift = float(min(max(d2.max() - 85.0, 0.0), d2.min() + 80.0))

    in_maps = []
    for c in range(NCORES):
        blk = slice(c * BLK, (c + 1) * BLK)
        in_maps.append(
            {
                "Th": Th,
                "Tl": Tl,
                "Tbh": Th[blk],
                "Tbl": Tl[blk],
                "Zh": Zh,
                "Zb": Z[blk],
            }
        )
    return in_maps, c_shift


def kernel(Z, W1, b1, W2, b2):
    in_maps, c_shift = prepare(Z, W1, b1, W2, b2)
    nc = _get_nc(c_shift)
    res = run_bass_kernel_spmd(nc, in_maps, list(range(NCORES)))
    return np.concatenate([res.results[c]["O"] for c in range(NCORES)], axis=0)


# revision 16
# speedup vs baseline: 1.3327x; 1.3327x over previous
"""Trainium2 Bass kernel for nn_DiffusionLayer (N=8192, D=128), 8-core SPMD.

Computation:
    t = relu(Z @ W1 + b1) @ W2 + b2      # [N, D]  (the MLP "transform")
    S = softmax(t @ t.T, axis=1)         # [N, N]
    out = Z + TAU * (S @ Z - Z)

Sharding (per the problem's hint): output rows split across 8 NeuronCores;
each core holds its 1024-row block and computes its S row-block against a
replicated transform_Z (t) and Z — flash-attention-style sequence
parallelism. t is computed once on the host (0.8% of total FLOPs — the
hint's "all-gathered transform_Z") and replicated to all cores as a bf16
hi+lo pair, which reconstructs to ~1e-7 relative accuracy; the O(N^2)
attention (99.2% of FLOPs) runs on device.

Device pipeline per core:
  - t^T via the DMA xbar transpose (2-byte only, hence the hi/lo pair) +
    one DVE add -> fp32r t^T in SBUF. No PE/PSUM involvement at all.
  - sim^T tiles [j-tile 128, i-chunk 256] = t[jt]^T' @ t_blk^T via fp32r
    matmuls (full speed, ~1.6e-4), grouped 6 j-tiles per 3-bank PSUM
    group so exp reads 1536-wide.
  - exp on ScalarE with a host-computed constant shift -C as the free
    activation bias (softmax is shift invariant; C keeps exp inside fp32
    range; row maxima >= ||t_i||^2 bound the denominator from below).
    Output E in bf16.
  - PV: E-slice^T @ [Zh | 1] in bf16, accumulated over all 64 j-tiles in
    PSUM; the appended ones column makes the softmax denominator fall
    out as output column 128. PV of chunk c-1 interleaves with sim of
    chunk c on the Tensor engine while ScalarE does exp.
  - normalize + residual on DVE per chunk, DMA out per chunk.
"""

import sys

sys.path.insert(0, "/opt/trn_rl_repo")

import numpy as np
import ml_dtypes
import orjson
from contextlib import ExitStack

import concourse.bass as bass
import concourse.tile as tile
from concourse import mybir
from concourse.bass_utils import run_bass_kernel_spmd

F32 = mybir.dt.float32
F32R = mybir.dt.float32r
BF16 = mybir.dt.bfloat16
BF = ml_dtypes.bfloat16

N, D = 8192, 128
NCORES = 8
BLK = N // NCORES  # 1024 rows per core
NT = N // 128  # 64 row tiles of full Z
NBT = BLK // 128  # 8 row tiles of the block
TAU = 0.1

CH = 256  # i-chunk width for sim/exp/PV
NCH = BLK // CH  # 4 chunks per core
GJ = 6  # j-tiles per sim PSUM group (3 banks)

# ---------------------------------------------------------------------------
# BIR post-pass: the walrus build in this image encodes at most one sync wait
# per instruction; Tile emits several on some instructions. Split excess
# waits onto preceding same-engine NoOp carriers (engines execute their
# stream in order, so this preserves semantics; NoOp stalls dispatch without
# flushing the engine pipeline).
_MAX_WAITS = 1


def _split_multiwaits(m: dict) -> bool:
    changed = False
    counter = [0]

    def fresh_name():
        counter[0] += 1
        return f"I-waitsplit-{counter[0]}"

    for fn in m.get("functions", []):
        for bb in fn.get("blocks", []):
            out = []
            for inst in bb.get("instructions", []):
                si = inst.get("sync_info") or {}
                waits = si.get("on_wait") or []
                if len(waits) > _MAX_WAITS:
                    changed = True
                    head, tail = waits[:-_MAX_WAITS], waits[-_MAX_WAITS:]
                    for i in range(0, len(head), _MAX_WAITS):
                        out.append(
                            {
                                "debug": inst.get("debug", 0),
                                "engine": inst["engine"],
                                "ins": [],
                                "name": fresh_name(),
                                "opcode": "NoOp",
                                "outs": [],
                                "sync_info": {
                                    "on_update": [],
                                    "on_wait": head[i : i + _MAX_WAITS],
                                },
                            }
                        )
                    si["on_wait"] = tail
                out.append(inst)
            bb["instructions"] = out
    return changed


def _patch_nc(nc):
    orig = nc.to_json_bytes

    def to_json_bytes_fixed():
        m = orjson.loads(orig())
        if _split_multiwaits(m):
            return orjson.dumps(m)
        return orig()

    nc.to_json_bytes = to_json_bytes_fixed
    return nc


# ---------------------------------------------------------------------------


def _build_nc(c_shift: float):
    nc = bass.Bass("TRN2", debug=False, num_devices=NCORES)

    Thd = nc.dram_tensor("Th", [N, D], BF16, kind="ExternalInput").ap()
    Tld = nc.dram_tensor("Tl", [N, D], BF16, kind="ExternalInput").ap()
    Tbhd = nc.dram_tensor("Tbh", [BLK, D], BF16, kind="ExternalInput").ap()
    Tbld = nc.dram_tensor("Tbl", [BLK, D], BF16, kind="ExternalInput").ap()
    Zhd = nc.dram_tensor("Zh", [N, D], BF16, kind="ExternalInput").ap()
    Zbd = nc.dram_tensor("Zb", [BLK, D], F32, kind="ExternalInput").ap()
    Od = nc.dram_tensor("O", [BLK, D], F32, kind="ExternalOutput").ap()

    Zhr = Zhd.rearrange("(t p) d -> p t d", p=128)  # [128, 64, 128]
    Zbr = Zbd.rearrange("(t p) d -> p t d", p=128)  # [128, 8, 128]
    Or = Od.rearrange("(t p) d -> p t d", p=128)

    with tile.TileContext(nc) as tc, ExitStack() as ctx:
        const = ctx.enter_context(tc.tile_pool(name="const", bufs=1))
        sb = ctx.enter_context(tc.tile_pool(name="sb", bufs=1))
        ebig = ctx.enter_context(tc.tile_pool(name="ebig", bufs=2))
        simps = ctx.enter_context(tc.tile_pool(name="simps", bufs=2, space="PSUM"))
        pvps = ctx.enter_context(tc.tile_pool(name="pvps", bufs=2, space="PSUM"))

        # ---- constants
        dummy = const.tile([128, 1], F32)
        nc.vector.memset(dummy[:], 0.0)
        dummy2 = const.tile([128, 1], F32)
        # preload the exp table set so the first real exp doesn't stall
        nc.scalar.activation(dummy2[:], dummy[:], mybir.ActivationFunctionType.Exp)
        cbias = const.tile([128, 1], F32)  # per-partition exp bias = -C
        nc.vector.memset(cbias[:], -c_shift)

        # ---- persistent SBUF tensors
        t_sb = sb.tile([128, N], F32R)  # t^T [d, N]
        tthi = sb.tile([128, N], BF16)
        ttlo = sb.tile([128, N], BF16)
        tb_sb = sb.tile([128, BLK], F32R)  # t_blk^T
        tbth = sb.tile([128, BLK], BF16)
        tbtl = sb.tile([128, BLK], BF16)
        zaug = sb.tile([128, NT, D + 1], BF16)  # [Zh | 1] row tiles
        zbn = sb.tile([128, NBT, 128], F32)  # Z block natural (residual)
        u_sb = sb.tile([128, NBT, D + 1], F32)  # unnormalized PV + denom
        o_sb = sb.tile([128, NBT, 128], F32)
        rec = sb.tile([128, NBT, 1], F32)

        # ---- all xbar transposes batched (Tile serializes on xbar-mode
        # transitions vs normal DMAs); block rows first — tb gates sim.
        nc.sync.dma_start_transpose(tbth[:], Tbhd)
        nc.sync.dma_start_transpose(tbtl[:], Tbld)
        for q in range(8):
            qs = slice(1024 * q, 1024 * (q + 1))
            nc.sync.dma_start_transpose(tthi[:, qs], Thd[qs, :])
            nc.sync.dma_start_transpose(ttlo[:, qs], Tld[qs, :])

        # ---- normal DMAs
        for q in range(4):
            nc.sync.dma_start(
                zaug[:, 16 * q : 16 * (q + 1), 0:D], Zhr[:, 16 * q : 16 * (q + 1), :]
            )
        nc.vector.memset(zaug[:, :, D : D + 1], 1.0)
        nc.sync.dma_start(zbn[:, 0:4, :], Zbr[:, 0:4, :])
        nc.sync.dma_start(zbn[:, 4:8, :], Zbr[:, 4:8, :])

        # ---- reconstruct fp32r t^T = hi + lo on DVE
        nc.vector.tensor_add(tb_sb[:], tbth[:], tbtl[:])
        for q in range(8):
            qs = slice(1024 * q, 1024 * (q + 1))
            nc.vector.tensor_add(t_sb[:, qs], tthi[:, qs], ttlo[:, qs])

        # ---- sim + exp + PV, chunked over i
        groups = []
        off = 0
        while off < NT:
            groups.append((off, min(GJ, NT - off)))
            off += GJ

        e_tiles = [None] * NCH

        def emit_sim_exp(c):
            ic = slice(CH * c, CH * (c + 1))
            e_sb = ebig.tile([128, NT, CH], BF16, tag="ebig", name=f"e_{c}")
            e_tiles[c] = e_sb
            for go, gn in groups:
                ps = simps.tile([128, GJ, CH], F32, tag="simps")
                for k in range(gn):
                    jt = go + k
                    nc.tensor.matmul(
                        ps[:, k, :],
                        t_sb[:, 128 * jt : 128 * (jt + 1)],
                        tb_sb[:, ic],
                        start=True,
                        stop=True,
                    )
                nc.scalar.activation(
                    e_sb[:, go : go + gn, :],
                    ps[:, 0:gn, :],
                    mybir.ActivationFunctionType.Exp,
                    bias=cbias[:],
                )

        def emit_pv(c):
            e_sb = e_tiles[c]
            for s in (2 * c, 2 * c + 1):
                si = (s % 2) * 128
                pv = pvps.tile([128, D + 1], F32, tag="ps", name=f"pv_{s}")
                for jt in range(NT):
                    nc.tensor.matmul(
                        pv[:],
                        e_sb[:, jt, si : si + 128],
                        zaug[:, jt, :],
                        start=(jt == 0),
                        stop=(jt == NT - 1),
                    )
                nc.vector.tensor_copy(u_sb[:, s, :], pv[:])

        def emit_out(c):
            # normalize + residual + store for chunk c's two row-slices
            sl = slice(2 * c, 2 * c + 2)
            nc.vector.reciprocal(rec[:, sl, :], u_sb[:, sl, D : D + 1])
            nc.vector.tensor_scalar_mul(rec[:, sl, :], rec[:, sl, :], TAU)
            for s in (2 * c, 2 * c + 1):
                nc.vector.tensor_scalar_mul(
                    u_sb[:, s, 0:D], u_sb[:, s, 0:D], rec[:, s, :]
                )
                nc.vector.scalar_tensor_tensor(
                    o_sb[:, s, :],
                    zbn[:, s, :],
                    1.0 - TAU,
                    u_sb[:, s, 0:D],
                    mybir.AluOpType.mult,
                    mybir.AluOpType.add,
                )
            nc.sync.dma_start(Or[:, sl, :], o_sb[:, sl, :])

        for c in range(NCH):
            emit_sim_exp(c)
            if c > 0:
                emit_pv(c - 1)
                emit_out(c - 1)
        emit_pv(NCH - 1)
        emit_out(NCH - 1)

    return _patch_nc(nc)


# ---------------------------------------------------------------------------

_CACHE = {}


def _get_nc(c_shift: float):
    key = round(float(c_shift), 3)
    if key not in _CACHE:
        _CACHE[key] = _build_nc(key)
    return _CACHE[key]


def prepare(Z, W1, b1, W2, b2):
    """Host-side prep: transform t, hi/lo splits, shift C, per-core maps."""
    Z = np.ascontiguousarray(np.asarray(Z, dtype=np.float32))
    W1 = np.ascontiguousarray(np.asarray(W1, dtype=np.float32))
    W2 = np.ascontiguousarray(np.asarray(W2, dtype=np.float32))
    b1 = np.asarray(b1, dtype=np.float32).reshape(1, D)
    b2 = np.asarray(b2, dtype=np.float32).reshape(1, D)

    t = (np.maximum(Z @ W1 + b1, 0.0) @ W2 + b2).astype(np.float32)
    Th = t.astype(BF)
    Tl = (t - Th.astype(np.float32)).astype(BF)
    Zh = Z.astype(BF)

    # constant softmax shift C: sim <= max||t||^2 (Cauchy-Schwarz), row
    # maxima >= diag = ||t_i||^2, so this window keeps exp in fp32 range
    # and the denominators in normal range.
    d2 = np.einsum("nd,nd->n", t, t)
    c_shift = float(min(max(d2.max() - 85.0, 0.0), d2.min() + 80.0))

    in_maps = []
    for c in range(NCORES):
        blk = slice(c * BLK, (c + 1) * BLK)
        in_maps.append(
            {
                "Th": Th,
                "Tl": Tl,
                "Tbh": Th[blk],
                "Tbl": Tl[blk],
                "Zh": Zh,
                "Zb": Z[blk],
            }
        )
    return in_maps, c_shift


def kernel(Z, W1, b1, W2, b2):
    in_maps, c_shift = prepare(Z, W1, b1, W2, b2)
    nc = _get_nc(c_shift)
    res = run_bass_kernel_spmd(nc, in_maps, list(range(NCORES)))
    return np.concatenate([res.results[c]["O"] for c in range(NCORES)], axis=0)


# revision 19
# speedup vs baseline: 1.5181x; 1.1391x over previous
"""Trainium2 Bass kernel for nn_DiffusionLayer (N=8192, D=128), 8-core SPMD.

Computation:
    t = relu(Z @ W1 + b1) @ W2 + b2      # [N, D]  (the MLP "transform")
    S = softmax(t @ t.T, axis=1)         # [N, N]
    out = Z + TAU * (S @ Z - Z)

Sharding (per the problem's hint): output rows split across 8 NeuronCores;
each core holds its 1024-row block and computes its S row-block against a
replicated transform_Z (t) and Z — flash-attention-style sequence
parallelism. t is computed once on the host (0.8% of total FLOPs — the
hint's "all-gathered transform_Z") and replicated to all cores as a bf16
hi+lo pair, which reconstructs to ~1e-7 relative accuracy; the O(N^2)
attention (99.2% of FLOPs) runs on device.

Device pipeline per core:
  - t^T via the DMA xbar transpose (2-byte only, hence the hi/lo pair) +
    one DVE add -> fp32r t^T in SBUF. No PE/PSUM involvement at all.
  - sim^T tiles [j-tile 128, i-chunk 256] = t[jt]^T' @ t_blk^T via fp32r
    matmuls (full speed, ~1.6e-4), grouped 6 j-tiles per 3-bank PSUM
    group so exp reads 1536-wide.
  - exp on ScalarE with a host-computed constant shift -C as the free
    activation bias (softmax is shift invariant; C keeps exp inside fp32
    range; row maxima >= ||t_i||^2 bound the denominator from below).
    Output E in bf16.
  - PV: E-slice^T @ [Zh | 1] in bf16, accumulated over all 64 j-tiles in
    PSUM; the appended ones column makes the softmax denominator fall
    out as output column 128. PV of chunk c-1 interleaves with sim of
    chunk c on the Tensor engine while ScalarE does exp.
  - normalize + residual on DVE per chunk, DMA out per chunk.
"""

import sys

sys.path.insert(0, "/opt/trn_rl_repo")

import numpy as np
import ml_dtypes
import orjson
from contextlib import ExitStack

import concourse.bass as bass
import concourse.tile as tile
from concourse import mybir
from concourse.bass_utils import run_bass_kernel_spmd

F32 = mybir.dt.float32
F32R = mybir.dt.float32r
BF16 = mybir.dt.bfloat16
BF = ml_dtypes.bfloat16

N, D = 8192, 128
NCORES = 8
BLK = N // NCORES  # 1024 rows per core
NT = N // 128  # 64 row tiles of full Z
NBT = BLK // 128  # 8 row tiles of the block
TAU = 0.1

CH = 256  # i-chunk width for sim/exp/PV
NCH = BLK // CH  # 4 chunks per core
GJ = 6  # j-tiles per sim PSUM group (3 banks)

# ---------------------------------------------------------------------------
# BIR post-pass: the walrus build in this image encodes at most one sync wait
# per instruction; Tile emits several on some instructions. Split excess
# waits onto preceding same-engine NoOp carriers (engines execute their
# stream in order, so this preserves semantics; NoOp stalls dispatch without
# flushing the engine pipeline).
_MAX_WAITS = 1


def _split_multiwaits(m: dict) -> bool:
    changed = False
    counter = [0]

    def fresh_name():
        counter[0] += 1
        return f"I-waitsplit-{counter[0]}"

    for fn in m.get("functions", []):
        for bb in fn.get("blocks", []):
            out = []
            for inst in bb.get("instructions", []):
                si = inst.get("sync_info") or {}
                waits = si.get("on_wait") or []
                if len(waits) > _MAX_WAITS:
                    changed = True
                    head, tail = waits[:-_MAX_WAITS], waits[-_MAX_WAITS:]
                    for i in range(0, len(head), _MAX_WAITS):
                        out.append(
                            {
                                "debug": inst.get("debug", 0),
                                "engine": inst["engine"],
                                "ins": [],
                                "name": fresh_name(),
                                "opcode": "NoOp",
                                "outs": [],
                                "sync_info": {
                                    "on_update": [],
                                    "on_wait": head[i : i + _MAX_WAITS],
                                },
                            }
                        )
                    si["on_wait"] = tail
                out.append(inst)
            bb["instructions"] = out
    return changed


def _patch_nc(nc):
    orig = nc.to_json_bytes

    def to_json_bytes_fixed():
        m = orjson.loads(orig())
        if _split_multiwaits(m):
            return orjson.dumps(m)
        return orig()

    nc.to_json_bytes = to_json_bytes_fixed
    return nc


# ---------------------------------------------------------------------------


def _build_nc(c_shift: float):
    nc = bass.Bass("TRN2", debug=False, num_devices=NCORES)

    Ttd = nc.dram_tensor("Tt", [D, N], F32, kind="ExternalInput").ap()
    Tbtd = nc.dram_tensor("Tbt", [D, BLK], F32, kind="ExternalInput").ap()
    Zad = nc.dram_tensor("Za", [N, D + 1], BF16, kind="ExternalInput").ap()
    Zbd = nc.dram_tensor("Zb", [BLK, D], F32, kind="ExternalInput").ap()
    Od = nc.dram_tensor("O", [BLK, D], F32, kind="ExternalOutput").ap()

    Zar = Zad.rearrange("(t p) e -> p t e", p=128)  # [128, 64, 129]
    Zbr = Zbd.rearrange("(t p) d -> p t d", p=128)  # [128, 8, 128]
    Or = Od.rearrange("(t p) d -> p t d", p=128)

    with tile.TileContext(nc) as tc, ExitStack() as ctx:
        const = ctx.enter_context(tc.tile_pool(name="const", bufs=1))
        sb = ctx.enter_context(tc.tile_pool(name="sb", bufs=1))
        ebig = ctx.enter_context(tc.tile_pool(name="ebig", bufs=2))
        simps = ctx.enter_context(tc.tile_pool(name="simps", bufs=2, space="PSUM"))
        pvps = ctx.enter_context(tc.tile_pool(name="pvps", bufs=2, space="PSUM"))

        # ---- constants
        dummy = const.tile([128, 1], F32)
        nc.vector.memset(dummy[:], 0.0)
        dummy2 = const.tile([128, 1], F32)
        # preload the exp table set so the first real exp doesn't stall
        nc.scalar.activation(dummy2[:], dummy[:], mybir.ActivationFunctionType.Exp)
        cbias = const.tile([128, 1], F32)  # per-partition exp bias = -C
        nc.vector.memset(cbias[:], -c_shift)

        # ---- persistent SBUF tensors
        t_sb = sb.tile([128, N], F32R)  # t^T [d, N]
        ttf = sb.tile([128, N], F32)  # fp32 staging for t^T
        tb_sb = sb.tile([128, BLK], F32R)  # t_blk^T
        tbtf = sb.tile([128, BLK], F32)
        zaug = sb.tile([128, NT, D + 1], BF16)  # [Zh | 1] row tiles
        zbn = sb.tile([128, NBT, 128], F32)  # Z block natural (residual)
        u_sb = sb.tile([128, NBT, D + 1], F32)  # unnormalized PV + denom
        o_sb = sb.tile([128, NBT, 128], F32)
        rec = sb.tile([128, NBT, 1], F32)

        # ---- loads: t^T arrives pre-transposed from the host (contiguous
        # per-partition rows — full-bandwidth plain DMAs). Block columns
        # first since tb gates every sim matmul. fp32 -> fp32r via DVE
        # cast (the rounding producer the fp32r matmul verifier wants).
        nc.sync.dma_start(tbtf[:], Tbtd)
        nc.vector.tensor_copy(tb_sb[:], tbtf[:])
        for q in range(8):
            qs = slice(1024 * q, 1024 * (q + 1))
            nc.sync.dma_start(ttf[:, qs], Ttd[:, qs])
            nc.vector.tensor_copy(t_sb[:, qs], ttf[:, qs])
        for q in range(4):
            nc.sync.dma_start(
                zaug[:, 16 * q : 16 * (q + 1), :], Zar[:, 16 * q : 16 * (q + 1), :]
            )
        nc.sync.dma_start(zbn[:, 0:4, :], Zbr[:, 0:4, :])
        nc.sync.dma_start(zbn[:, 4:8, :], Zbr[:, 4:8, :])

        # ---- sim + exp + PV, chunked over i
        groups = []
        off = 0
        while off < NT:
            groups.append((off, min(GJ, NT - off)))
            off += GJ

        e_tiles = [None] * NCH

        def emit_sim_exp(c):
            ic = slice(CH * c, CH * (c + 1))
            e_sb = ebig.tile([128, NT, CH], BF16, tag="ebig", name=f"e_{c}")
            e_tiles[c] = e_sb
            for go, gn in groups:
                ps = simps.tile([128, GJ, CH], F32, tag="simps")
                for k in range(gn):
                    jt = go + k
                    nc.tensor.matmul(
                        ps[:, k, :],
                        t_sb[:, 128 * jt : 128 * (jt + 1)],
                        tb_sb[:, ic],
                        start=True,
                        stop=True,
                    )
                nc.scalar.activation(
                    e_sb[:, go : go + gn, :],
                    ps[:, 0:gn, :],
                    mybir.ActivationFunctionType.Exp,
                    bias=cbias[:],
                )

        def emit_pv(c):
            e_sb = e_tiles[c]
            for s in (2 * c, 2 * c + 1):
                si = (s % 2) * 128
                pv = pvps.tile([128, D + 1], F32, tag="ps", name=f"pv_{s}")
                for jt in range(NT):
                    nc.tensor.matmul(
                        pv[:],
                        e_sb[:, jt, si : si + 128],
                        zaug[:, jt, :],
                        start=(jt == 0),
                        stop=(jt == NT - 1),
                    )
                nc.vector.tensor_copy(u_sb[:, s, :], pv[:])

        def emit_out(c):
            # normalize + residual + store for chunk c's two row-slices
            sl = slice(2 * c, 2 * c + 2)
            nc.vector.reciprocal(rec[:, sl, :], u_sb[:, sl, D : D + 1])
            nc.vector.tensor_scalar_mul(rec[:, sl, :], rec[:, sl, :], TAU)
            for s in (2 * c, 2 * c + 1):
                nc.vector.tensor_scalar_mul(
                    u_sb[:, s, 0:D], u_sb[:, s, 0:D], rec[:, s, :]
                )
                nc.vector.scalar_tensor_tensor(
                    o_sb[:, s, :],
                    zbn[:, s, :],
                    1.0 - TAU,
                    u_sb[:, s, 0:D],
                    mybir.AluOpType.mult,
                    mybir.AluOpType.add,
                )
            nc.sync.dma_start(Or[:, sl, :], o_sb[:, sl, :])

        for c in range(NCH):
            emit_sim_exp(c)
            if c > 0:
                emit_pv(c - 1)
                emit_out(c - 1)
        emit_pv(NCH - 1)
        emit_out(NCH - 1)

    return _patch_nc(nc)


# ---------------------------------------------------------------------------

_CACHE = {}


def _get_nc(c_shift: float):
    key = round(float(c_shift), 3)
    if key not in _CACHE:
        _CACHE[key] = _build_nc(key)
    return _CACHE[key]


def prepare(Z, W1, b1, W2, b2):
    """Host-side prep: transform t, hi/lo splits, shift C, per-core maps."""
    Z = np.ascontiguousarray(np.asarray(Z, dtype=np.float32))
    W1 = np.ascontiguousarray(np.asarray(W1, dtype=np.float32))
    W2 = np.ascontiguousarray(np.asarray(W2, dtype=np.float32))
    b1 = np.asarray(b1, dtype=np.float32).reshape(1, D)
    b2 = np.asarray(b2, dtype=np.float32).reshape(1, D)

    t = (np.maximum(Z @ W1 + b1, 0.0) @ W2 + b2).astype(np.float32)
    Tt = np.ascontiguousarray(t.T)
    Za = np.concatenate([Z, np.ones((N, 1), np.float32)], axis=1).astype(BF)

    # constant softmax shift C: sim <= max||t||^2 (Cauchy-Schwarz), row
    # maxima >= diag = ||t_i||^2, so this window keeps exp in fp32 range
    # and the denominators in normal range.
    d2 = np.einsum("nd,nd->n", t, t)
    c_shift = float(min(max(d2.max() - 85.0, 0.0), d2.min() + 80.0))

    in_maps = []
    for c in range(NCORES):
        blk = slice(c * BLK, (c + 1) * BLK)
        in_maps.append(
            {
                "Tt": Tt,
                "Tbt": np.ascontiguousarray(Tt[:, blk]),
                "Za": Za,
                "Zb": Z[blk],
            }
        )
    return in_maps, c_shift


def kernel(Z, W1, b1, W2, b2):
    in_maps, c_shift = prepare(Z, W1, b1, W2, b2)
    nc = _get_nc(c_shift)
    res = run_bass_kernel_spmd(nc, in_maps, list(range(NCORES)))
    return np.concatenate([res.results[c]["O"] for c in range(NCORES)], axis=0)
